# revision 11
# baseline (speedup 1.0000x reference)
"""AGNN (3-layer) distributed Bass kernel for 8 Trainium2 NeuronCores.

Sharding: nodes partitioned into 8 contiguous destination shards; edges routed
to the core owning their destination, sorted by destination block (128 dsts),
split into A/B groups by src < 32768 (int16 gather index limit), padded to
128-edge tiles. Per layer: local dense + L2-normalize on the shard, AllGather
of bf16 h-rows into a per-core DRAM table (i-major shard layout so the shard
dump is one contiguous DMA), then per edge tile:
  - h[src] rows via SWDGE dma_gather (4 queues, <=896 idxs per gather)
  - hn[dst] rows expanded by a one-hot matmul on TensorE
  - alpha_e = (||hn_dst + h_src||^2 - 1 - ||h_src||^2) / (2 ||h_src||)
  - softmax scalars on per-tile [128,1] columns
  - message + denominator via an ex-scaled one-hot segment matmul into PSUM
"""
import sys

sys.path.insert(0, "/opt/trn_rl_repo")
sys.path.insert(0, "/opt/trn_rl_repo/concourse")

import numpy as np
import ml_dtypes

N_NODES = 50000
D = 128
D_OUT = 64
NCORES = 8
SPLIT = 32768      # int16 gather index limit
GMAX_TILES = 7     # max tiles (of 128 idxs) per dma_gather (single-packet limit)

SHARD = N_NODES // NCORES
BLK = 128
NBLK = (SHARD + BLK - 1) // BLK
SHARD_PAD = NBLK * BLK


def _set_dims(n_nodes, d_out=64):
    global N_NODES, D_OUT, SHARD, NBLK, SHARD_PAD
    N_NODES = n_nodes
    D_OUT = d_out
    SHARD = N_NODES // NCORES
    NBLK = (SHARD + BLK - 1) // BLK
    SHARD_PAD = NBLK * BLK


def _table_row(j):
    """Global table row for node j (shards dumped i-major: row = i*NBLK + b)."""
    c = j // SHARD
    l = j - c * SHARD
    return c * SHARD_PAD + (l % BLK) * NBLK + (l // BLK)


def _wrap_idx(idx):
    """int16 idx stream -> [128, n/16] wrapped tile (pos k = col*16 + row)."""
    n = len(idx)
    assert n % 16 == 0
    w = idx.reshape(n // 16, 16).T.astype(np.int16)
    return np.tile(w, (8, 1))


def preprocess(x, edge_index):
    src = np.asarray(edge_index[0], dtype=np.int64)
    dst = np.asarray(edge_index[1], dtype=np.int64)
    loops = np.arange(N_NODES, dtype=np.int64)
    src = np.concatenate([src, loops])
    dst = np.concatenate([dst, loops])

    core = dst // SHARD
    dstl = dst - core * SHARD
    blk = dstl // BLK
    srow = _table_row(src)  # gather row in the i-major table
    grp = (srow >= SPLIT).astype(np.int64)

    counts = np.zeros((NCORES, NBLK, 2), dtype=np.int64)
    np.add.at(counts, (core, blk, grp), 1)
    tiles = (counts + BLK - 1) // BLK
    tiles_shared = tiles.max(axis=0)  # [NBLK, 2]

    order = np.lexsort((srow, grp, blk, core))
    srow_s, dstl_s, core_s, blk_s, grp_s = (
        srow[order], dstl[order], core[order], blk[order], grp[order])

    # per chunk (= one block): [A tiles, B tiles]
    chunk_plan = []
    for b in range(NBLK):
        chunk_plan.append(
            [(b, 0, int(tiles_shared[b, 0])), (b, 1, int(tiles_shared[b, 1]))])
    sched = {"tiles": tiles_shared, "chunk_plan": chunk_plan}

    start = np.searchsorted(core_s, np.arange(NCORES))
    end = np.searchsorted(core_s, np.arange(NCORES) + 1)

    per_core = []
    for cid in range(NCORES):
        s0, s1 = start[cid], end[cid]
        csrow, cdstl, cblk, cgrp = (
            srow_s[s0:s1], dstl_s[s0:s1], blk_s[s0:s1], grp_s[s0:s1])
        key = cblk * 2 + cgrp
        idxA, idxB, dstl_tiles = [], [], []
        for plan in chunk_plan:
            for b, g, nt in plan:
                k = b * 2 + g
                i0 = np.searchsorted(key, k)
                i1 = np.searchsorted(key, k + 1)
                cnt = i1 - i0
                pad = nt * BLK - cnt
                s_idx = csrow[i0:i1]
                if g == 1:
                    s_idx = s_idx - SPLIT
                s_idx = np.concatenate([s_idx, np.zeros(pad, np.int64)])
                d_loc = np.concatenate(
                    [cdstl[i0:i1] - b * BLK, -np.ones(pad, np.int64)])
                (idxA if g == 0 else idxB).append(s_idx)
                dstl_tiles.append(d_loc.reshape(nt, BLK))
        dstl_all = np.concatenate(dstl_tiles, axis=0)
        onehot = (dstl_all[:, None, :] == np.arange(128)[None, :, None])
        per_core.append({
            "idxA": np.concatenate(idxA).astype(np.int16),
            "idxB": np.concatenate(idxB).astype(np.int16),
            "dstl": dstl_all,
            "snm": np.ascontiguousarray(
                onehot.transpose(1, 0, 2)).astype(ml_dtypes.bfloat16),
        })
    return sched, per_core


def build_kernel(sched, n_tiles_total, nA, nB):
    from concourse import bass, bacc, tile, mybir

    F32 = mybir.dt.float32
    BF16 = mybir.dt.bfloat16
    I16 = mybir.dt.int16
    AL = mybir.AluOpType
    AF = mybir.ActivationFunctionType

    nc = bacc.Bacc(None, target_bir_lowering=False, num_swdge_queues=4)

    xT_in = nc.dram_tensor("xT", [D, SHARD_PAD], BF16, kind="ExternalInput")
    W_in = [nc.dram_tensor(f"w{l}", [D, D], BF16, kind="ExternalInput") for l in range(3)]
    b_in = [nc.dram_tensor(f"b{l}", [D, 1], F32, kind="ExternalInput") for l in range(3)]
    idxA_in = nc.dram_tensor("idxA", [128, nA // 16], I16, kind="ExternalInput")
    idxB_in = nc.dram_tensor("idxB", [128, nB // 16], I16, kind="ExternalInput")
    dstlc_in = nc.dram_tensor("dstlc", [128, n_tiles_total], F32, kind="ExternalInput")
    dstlr_in = nc.dram_tensor("dstlr", [1, n_tiles_total * BLK], BF16, kind="ExternalInput")
    iota_row_in = nc.dram_tensor("iota_row", [128, 128], BF16, kind="ExternalInput")
    iota_col_in = nc.dram_tensor("iota_col", [128, 1], F32, kind="ExternalInput")
    ones_row_in = nc.dram_tensor("ones_row", [1, 128], BF16, kind="ExternalInput")
    ones_col_in = nc.dram_tensor("ones_col", [128, 1], BF16, kind="ExternalInput")
    ident_in = nc.dram_tensor("ident", [128, 128], BF16, kind="ExternalInput")
    snm_in = nc.dram_tensor("snm", [128, n_tiles_total, 128], BF16, kind="ExternalInput")
    out_dram = nc.dram_tensor("out", [SHARD_PAD, D_OUT], F32, kind="ExternalOutput")

    shard_dram = nc.dram_tensor("shard_d", [SHARD_PAD, D], BF16, kind="Internal")
    table = nc.dram_tensor(
        "table_d", [NCORES * SHARD_PAD, D], BF16, kind="Internal",
        addr_space="Shared")

    chunk_plan = sched["chunk_plan"]
    chunk_tiles = [sum(nt for _, _, nt in plan) for plan in chunk_plan]
    max_chunk_tiles = max(chunk_tiles)
    n_table_rows = NCORES * SHARD_PAD

    with tile.TileContext(nc) as tc:
        with (
            tc.tile_pool(name="const", bufs=1) as cpool,
            tc.tile_pool(name="state", bufs=1) as spool,
            tc.tile_pool(name="hTp", bufs=1) as hTp,
            tc.tile_pool(name="gbuf", bufs=2) as gpool,
            tc.tile_pool(name="work", bufs=3) as wpool,
            tc.tile_pool(name="cols", bufs=2) as colpool,
            tc.tile_pool(name="psBig", bufs=1, space="PSUM") as psBig,
            tc.tile_pool(name="psO", bufs=1, space="PSUM") as psO,
            tc.tile_pool(name="psD", bufs=1, space="PSUM") as psD,
        ):
            iota_row = cpool.tile([128, 128], BF16)
            nc.sync.dma_start(iota_row[:], iota_row_in[:])
            iota_col = cpool.tile([128, 1], F32)
            nc.sync.dma_start(iota_col[:], iota_col_in[:])
            ones_row = cpool.tile([1, 128], BF16)
            nc.sync.dma_start(ones_row[:], ones_row_in[:])
            ones_col = cpool.tile([128, 1], BF16)
            nc.sync.dma_start(ones_col[:], ones_col_in[:])
            ident = cpool.tile([128, 128], BF16)
            nc.sync.dma_start(ident[:], ident_in[:])
            idxA_sb = cpool.tile([128, nA // 16], I16)
            nc.sync.dma_start(idxA_sb[:], idxA_in[:])
            idxB_sb = cpool.tile([128, nB // 16], I16)
            nc.sync.dma_start(idxB_sb[:], idxB_in[:])
            dstlc = cpool.tile([128, n_tiles_total], F32)
            nc.sync.dma_start(dstlc[:], dstlc_in[:])
            Wt, bt = [], []
            for l in range(3):
                w = cpool.tile([D, D], BF16, tag=f"w{l}")
                nc.sync.dma_start(w[:], W_in[l][:])
                Wt.append(w)
                b = cpool.tile([D, 1], F32, tag=f"b{l}")
                nc.sync.dma_start(b[:], b_in[l][:])
                bt.append(b)

            xT = spool.tile([D, SHARD_PAD], BF16, tag="xT")
            nc.sync.dma_start(xT[:], xT_in[:])
            h_nm = spool.tile([128, NBLK, D], BF16, tag="h_nm")
            hnD = spool.tile([128, NBLK, D], BF16, tag="hnD")
            x_nm = spool.tile([128, NBLK, D], BF16, tag="x_nm")

            for layer in range(3):
                # ===== dense =====
                hT = hTp.tile([D, SHARD_PAD], BF16, tag="hT")
                for j in range(0, SHARD_PAD, 512):
                    jn = min(512, SHARD_PAD - j)
                    w_ps = psBig.tile([128, 512], F32, tag="big")
                    nc.tensor.matmul(w_ps[:, 0:jn], Wt[layer][:], xT[:, j:j + jn],
                                     start=True, stop=True)
                    nc.vector.tensor_scalar(
                        hT[:, j:j + jn], w_ps[:, 0:jn], bt[layer][:], None, AL.add)
                for b in range(NBLK):
                    t_ps = psBig.tile([128, 128], BF16, tag="big")
                    nc.tensor.transpose(t_ps[:], hT[:, b * 128:(b + 1) * 128], ident[:])
                    nc.vector.tensor_copy(h_nm[:, b, :], t_ps[:])
                n2c = colpool.tile([128, NBLK], F32, tag="n2c")
                for b in range(NBLK):
                    scr0 = wpool.tile([128, D], BF16, tag="scr0")
                    nc.scalar.activation(scr0[:], h_nm[:, b, :], AF.Square,
                                         accum_out=n2c[:, b:b + 1])
                nc.vector.tensor_scalar_max(n2c[:], n2c[:], 1e-24)
                nrm = colpool.tile([128, NBLK], F32, tag="nrm")
                nc.scalar.activation(nrm[:], n2c[:], AF.Sqrt)
                rnrm = colpool.tile([128, NBLK], F32, tag="rnrm")
                nc.vector.reciprocal(rnrm[:], nrm[:])
                for b in range(NBLK):
                    nc.vector.tensor_scalar(
                        hnD[:, b, :], h_nm[:, b, :], rnrm[:, b:b + 1], None, AL.mult)

                # ===== allgather h shard (i-major contiguous dump) =====
                nc.sync.dma_start(
                    shard_dram[:].rearrange("(i b) d -> i (b d)", i=128),
                    h_nm[:].rearrange("i b d -> i (b d)"))
                nc.gpsimd.collective_compute(
                    "AllGather",
                    mybir.AluOpType.bypass,
                    replica_groups=[list(range(NCORES))],
                    ins=[shard_dram[:].opt()],
                    outs=[table[:].opt()],
                )

                # ===== edge phase =====
                tile_off = 0
                offA = 0
                offB = 0
                qsel = 0
                for ci, plan in enumerate(chunk_plan):
                    T = chunk_tiles[ci]
                    TA = plan[0][2]
                    blk_id = plan[0][0]
                    G = gpool.tile([128, max_chunk_tiles, D], BF16, tag="G")

                    def issue_gathers(idx_sb, off, t0, ntiles, base_row):
                        nonlocal qsel
                        done = 0
                        while done < ntiles:
                            nt = min(GMAX_TILES, ntiles - done)
                            nidx = nt * 128
                            nc.gpsimd.dma_gather(
                                G[:, t0 + done:t0 + done + nt, :],
                                table[base_row:n_table_rows, :],
                                idx_sb[:, (off + done * 128) // 16:
                                       (off + (done + nt) * 128) // 16],
                                nidx, nidx, D,
                                single_packet=True,
                                queue_num=qsel % 4,
                            )
                            qsel += 1
                            done += nt

                    issue_gathers(idxA_sb, offA, 0, TA, 0)
                    if T - TA > 0 and n_table_rows > SPLIT:
                        issue_gathers(idxB_sb, offB, TA, T - TA, SPLIT)
                    offA += TA * 128
                    offB += (T - TA) * 128

                    Q = colpool.tile([128, max_chunk_tiles], F32, tag="Q")
                    N2 = colpool.tile([128, max_chunk_tiles], F32, tag="N2")

                    snm_sb = gpool.tile([128, max_chunk_tiles, 128], BF16, tag="snm")
                    nc.sync.dma_start(
                        snm_sb[:, 0:T, :], snm_in[:, tile_off:tile_off + T, :])
                    hd_ps = psBig.tile([128, max_chunk_tiles * 128], F32, tag="big")
                    for t in range(T):
                        nc.tensor.matmul(
                            hd_ps[:, t * 128:(t + 1) * 128], snm_sb[:, t, :],
                            hnD[:, blk_id, :], start=True, stop=True)
                    U = gpool.tile([128, max_chunk_tiles, 128], BF16, tag="U")
                    nc.vector.tensor_tensor(
                        U[:, 0:T, :].rearrange("p t d -> p (t d)"),
                        hd_ps[:, 0:T * 128],
                        G[:, 0:T, :].rearrange("p t d -> p (t d)"), AL.add)
                    squ = gpool.tile([128, max_chunk_tiles, 128], BF16, tag="squ")
                    nc.scalar.activation(
                        squ[:, 0:T, :].rearrange("p t d -> p (t d)"),
                        U[:, 0:T, :].rearrange("p t d -> p (t d)"), AF.Square)
                    nc.vector.tensor_reduce(
                        Q[:, 0:T], squ[:, 0:T, :], mybir.AxisListType.X, AL.add)
                    sqg = gpool.tile([128, max_chunk_tiles, 128], BF16, tag="sqg")
                    nc.scalar.activation(
                        sqg[:, 0:T, :].rearrange("p t d -> p (t d)"),
                        G[:, 0:T, :].rearrange("p t d -> p (t d)"), AF.Square)
                    nc.vector.tensor_reduce(
                        N2[:, 0:T], sqg[:, 0:T, :], mybir.AxisListType.X, AL.add)

                    # alpha = (Q - N2 - 1) * 0.5 / sqrt(N2); EX = exp(alpha)
                    n2cl = colpool.tile([128, max_chunk_tiles], F32, tag="n2cl")
                    nc.vector.tensor_scalar_max(n2cl[:, 0:T], N2[:, 0:T], 1e-24)
                    nsr = colpool.tile([128, max_chunk_tiles], F32, tag="nsr")
                    nc.scalar.activation(nsr[:, 0:T], n2cl[:, 0:T], AF.Sqrt)
                    rn = colpool.tile([128, max_chunk_tiles], F32, tag="rn")
                    nc.vector.reciprocal(rn[:, 0:T], nsr[:, 0:T])
                    al1 = colpool.tile([128, max_chunk_tiles], F32, tag="al1")
                    nc.vector.tensor_tensor(al1[:, 0:T], Q[:, 0:T], N2[:, 0:T],
                                            AL.subtract)
                    nc.vector.tensor_scalar(al1[:, 0:T], al1[:, 0:T], -1.0, 0.5,
                                            AL.add, AL.mult)
                    nc.vector.tensor_tensor(al1[:, 0:T], al1[:, 0:T], rn[:, 0:T],
                                            AL.mult)
                    EX = colpool.tile([128, max_chunk_tiles], F32, tag="EX")
                    nc.scalar.activation(EX[:, 0:T], al1[:, 0:T], AF.Exp)

                    blk_ps = psO.tile([128, 128], F32, tag="blk")
                    den_ps = psD.tile([128, 1], F32, tag="den")
                    for t in range(T):
                        W_T = wpool.tile([128, 128], BF16, tag="W_T")
                        nc.vector.tensor_scalar(
                            W_T[:], iota_row[:],
                            dstlc[:, tile_off + t:tile_off + t + 1],
                            EX[:, t:t + 1], AL.is_equal, AL.mult)
                        first = t == 0
                        last = t == T - 1
                        nc.tensor.matmul(blk_ps[:], W_T[:], G[:, t, :],
                                         start=first, stop=last)
                        nc.tensor.matmul(den_ps[:], W_T[:], ones_col[:],
                                         start=first, stop=last)

                    den = colpool.tile([128, 1], F32, tag="den")
                    nc.vector.tensor_scalar_max(den[:], den_ps[:], 1e-30)
                    rd = colpool.tile([128, 1], F32, tag="rd")
                    nc.vector.reciprocal(rd[:], den[:])
                    if layer < 2:
                        nc.scalar.activation(
                            x_nm[:, blk_id, :], blk_ps[:, 0:128], AF.Relu,
                            scale=rd[:])
                    else:
                        z = wpool.tile([128, D_OUT], F32, tag="z")
                        nc.scalar.activation(
                            z[:], blk_ps[:, 0:D_OUT], AF.Copy, scale=rd[:])
                        mx = colpool.tile([128, 1], F32, tag="mx")
                        nc.vector.tensor_reduce(
                            mx[:], z[:], mybir.AxisListType.X, AL.max)
                        nmx = colpool.tile([128, 1], F32, tag="nmx")
                        nc.vector.tensor_scalar_mul(nmx[:], mx[:], -1.0)
                        escr = wpool.tile([128, D_OUT], F32, tag="escr")
                        se = colpool.tile([128, 1], F32, tag="se")
                        nc.scalar.activation(escr[:], z[:], AF.Exp,
                                             bias=nmx[:], accum_out=se[:])
                        lse = colpool.tile([128, 1], F32, tag="lse")
                        nc.scalar.activation(lse[:], se[:], AF.Ln)
                        zo = wpool.tile([128, D_OUT], F32, tag="zo")
                        nc.vector.tensor_scalar(
                            zo[:], z[:], mx[:], lse[:], AL.subtract, AL.subtract)
                        nc.sync.dma_start(
                            out_dram[:].rearrange("(b i) d -> b i d", i=128)[blk_id],
                            zo[:])
                    tile_off += T

                if layer < 2:
                    for b in range(NBLK):
                        t_ps = psBig.tile([128, 128], BF16, tag="big")
                        nc.tensor.transpose(t_ps[:], x_nm[:, b, :], ident[:])
                        nc.vector.tensor_copy(xT[:, b * 128:(b + 1) * 128], t_ps[:])

    nc.finalize()
    return nc


def prepare(x, edge_index, W1, b1, W2, b2, W3, b3):
    x = np.asarray(x, dtype=np.float32)
    sched, per_core = preprocess(x, edge_index)
    tiles = sched["tiles"]
    n_tiles_total = int(tiles.sum())
    nA = int(tiles[:, 0].sum()) * BLK
    nB = max(int(tiles[:, 1].sum()) * BLK, 128)

    nc = build_kernel(sched, n_tiles_total, nA, nB)

    def wt_pad(W):
        W = np.asarray(W, np.float32)
        dout, din = W.shape
        M = np.zeros((D, D), np.float32)
        M[:din, :dout] = W.T
        return M.astype(ml_dtypes.bfloat16)

    def b_pad(b):
        b = np.asarray(b, np.float32)
        M = np.zeros((D, 1), np.float32)
        M[: len(b), 0] = b
        return M

    Ws = [wt_pad(W1), wt_pad(W2), wt_pad(W3)]
    bs = [b_pad(b1), b_pad(b2), b_pad(b3)]

    iota_row = np.tile(np.arange(128, dtype=np.float32)[None, :], (128, 1)).astype(
        ml_dtypes.bfloat16)
    iota_col = np.arange(128, dtype=np.float32)[:, None]
    ones_row = np.ones((1, 128), ml_dtypes.bfloat16)
    ones_col = np.ones((128, 1), ml_dtypes.bfloat16)
    ident = np.eye(128, dtype=np.float32).astype(ml_dtypes.bfloat16)

    in_maps = []
    for cid in range(NCORES):
        pc = per_core[cid]
        xs = np.zeros((D, SHARD_PAD), np.float32)
        xs[:, :SHARD] = x[cid * SHARD:(cid + 1) * SHARD].T
        dstl = pc["dstl"]
        idxB = pc["idxB"]
        if len(idxB) < nB:
            idxB = np.concatenate([idxB, np.zeros(nB - len(idxB), np.int16)])
        m = {
            "xT": xs.astype(ml_dtypes.bfloat16),
            "w0": Ws[0], "w1": Ws[1], "w2": Ws[2],
            "b0": bs[0], "b1": bs[1], "b2": bs[2],
            "idxA": _wrap_idx(pc["idxA"]),
            "idxB": _wrap_idx(idxB),
            "dstlc": np.ascontiguousarray(dstl.T.astype(np.float32)),
            "dstlr": dstl.reshape(1, -1).astype(np.float32).astype(
                ml_dtypes.bfloat16),
            "snm": pc["snm"],
            "iota_row": iota_row,
            "iota_col": iota_col,
            "ones_row": ones_row,
            "ones_col": ones_col,
            "ident": ident,
        }
        in_maps.append(m)
    return nc, in_maps


def _ensure_ntff_shim():
    """bass_utils trace=True needs antenv.axon_hooks; provide it if absent."""
    import types, importlib
    try:
        importlib.import_module("antenv.axon_hooks")
        return
    except Exception:
        pass
    mod = types.ModuleType("antenv.axon_hooks")
    _h = [None]
    mod.set_axon_ntff_profile_hook = lambda h: _h.__setitem__(0, h)
    mod.get_axon_ntff_profile_hook = lambda: _h[0]
    sys.modules["antenv.axon_hooks"] = mod
    try:
        import antenv
        antenv.axon_hooks = mod
        from trn_agent_boot.trn_boot import _ntff_profile_via_ctypes
        mod.set_axon_ntff_profile_hook(
            _ntff_profile_via_ctypes("/opt/axon/libaxon_pjrt.so"))
    except Exception:
        pass


def _axon_reset():
    try:
        import ctypes, jax
        jax.devices()
        lib = ctypes.CDLL("/opt/axon/libaxon_pjrt.so")
        lib.axon_reset.restype = ctypes.c_int64
        lib.axon_reset()
    except Exception:
        pass


def kernel(x, edge_index, W1, b1, W2, b2, W3, b3):
    _ensure_ntff_shim()
    from concourse import bass_utils

    nc, in_maps = prepare(x, edge_index, W1, b1, W2, b2, W3, b3)
    _axon_reset()
    res = None
    for attempt in range(3):
        try:
            res = bass_utils.run_bass_kernel_spmd(
                nc, in_maps, core_ids=list(range(NCORES)),
                trace=(attempt == 0))
            break
        except Exception:
            if attempt == 2:
                raise
            _axon_reset()
    kernel.last_exec_time_ns = res.exec_time_ns

    out = np.zeros((N_NODES, D_OUT), np.float32)
    for cid in range(NCORES):
        out[cid * SHARD:(cid + 1) * SHARD] = res.results[cid]["out"][:SHARD]
    return out


kernel.last_exec_time_ns = None


# revision 13
# speedup vs baseline: 1.1493x; 1.1493x over previous
"""AGNN (3-layer) distributed Bass kernel for 8 Trainium2 NeuronCores.

Sharding: nodes partitioned into 8 contiguous destination shards; edges routed
to the core owning their destination, sorted by destination block (128 dsts),
split into A/B groups by src < 32768 (int16 gather index limit), padded to
128-edge tiles. Per layer: local dense + L2-normalize on the shard, AllGather
of bf16 h-rows into a per-core DRAM table (i-major shard layout so the shard
dump is one contiguous DMA), then per edge tile:
  - h[src] rows via SWDGE dma_gather (4 queues, <=896 idxs per gather)
  - hn[dst] rows expanded by a one-hot matmul on TensorE
  - alpha_e = (||hn_dst + h_src||^2 - 1 - ||h_src||^2) / (2 ||h_src||)
  - softmax scalars on per-tile [128,1] columns
  - message + denominator via an ex-scaled one-hot segment matmul into PSUM
"""
import sys

sys.path.insert(0, "/opt/trn_rl_repo")
sys.path.insert(0, "/opt/trn_rl_repo/concourse")

import numpy as np
import ml_dtypes

N_NODES = 50000
D = 128
D_OUT = 64
NCORES = 8
SPLIT = 32768      # int16 gather index limit
GMAX_TILES = 7     # max tiles (of 128 idxs) per dma_gather (single-packet limit)

SHARD = N_NODES // NCORES
BLK = 128
NBLK = (SHARD + BLK - 1) // BLK
SHARD_PAD = NBLK * BLK


def _set_dims(n_nodes, d_out=64):
    global N_NODES, D_OUT, SHARD, NBLK, SHARD_PAD
    N_NODES = n_nodes
    D_OUT = d_out
    SHARD = N_NODES // NCORES
    NBLK = (SHARD + BLK - 1) // BLK
    SHARD_PAD = NBLK * BLK


def _table_row(j):
    """Global table row for node j (shards dumped i-major: row = i*NBLK + b)."""
    c = j // SHARD
    l = j - c * SHARD
    return c * SHARD_PAD + (l % BLK) * NBLK + (l // BLK)


def _wrap_idx(idx):
    """int16 idx stream -> [128, n/16] wrapped tile (pos k = col*16 + row)."""
    n = len(idx)
    assert n % 16 == 0
    w = idx.reshape(n // 16, 16).T.astype(np.int16)
    return np.tile(w, (8, 1))


def preprocess(x, edge_index):
    src = np.asarray(edge_index[0], dtype=np.int64)
    dst = np.asarray(edge_index[1], dtype=np.int64)
    loops = np.arange(N_NODES, dtype=np.int64)
    src = np.concatenate([src, loops])
    dst = np.concatenate([dst, loops])

    core = dst // SHARD
    dstl = dst - core * SHARD
    blk = dstl // BLK
    srow = _table_row(src)  # gather row in the i-major table
    grp = (srow >= SPLIT).astype(np.int64)

    counts = np.zeros((NCORES, NBLK, 2), dtype=np.int64)
    np.add.at(counts, (core, blk, grp), 1)
    tiles = (counts + BLK - 1) // BLK
    tiles_shared = tiles.max(axis=0)  # [NBLK, 2]

    order = np.lexsort((srow, grp, blk, core))
    srow_s, dstl_s, core_s, blk_s, grp_s = (
        srow[order], dstl[order], core[order], blk[order], grp[order])

    # per chunk (= one block): [A tiles, B tiles]
    chunk_plan = []
    for b in range(NBLK):
        chunk_plan.append(
            [(b, 0, int(tiles_shared[b, 0])), (b, 1, int(tiles_shared[b, 1]))])
    sched = {"tiles": tiles_shared, "chunk_plan": chunk_plan}

    start = np.searchsorted(core_s, np.arange(NCORES))
    end = np.searchsorted(core_s, np.arange(NCORES) + 1)

    per_core = []
    for cid in range(NCORES):
        s0, s1 = start[cid], end[cid]
        csrow, cdstl, cblk, cgrp = (
            srow_s[s0:s1], dstl_s[s0:s1], blk_s[s0:s1], grp_s[s0:s1])
        key = cblk * 2 + cgrp
        idxA, idxB, dstl_tiles = [], [], []
        for plan in chunk_plan:
            for b, g, nt in plan:
                k = b * 2 + g
                i0 = np.searchsorted(key, k)
                i1 = np.searchsorted(key, k + 1)
                cnt = i1 - i0
                pad = nt * BLK - cnt
                s_idx = csrow[i0:i1]
                if g == 1:
                    s_idx = s_idx - SPLIT
                s_idx = np.concatenate([s_idx, np.zeros(pad, np.int64)])
                d_loc = np.concatenate(
                    [cdstl[i0:i1] - b * BLK, -np.ones(pad, np.int64)])
                (idxA if g == 0 else idxB).append(s_idx)
                dstl_tiles.append(d_loc.reshape(nt, BLK))
        dstl_all = np.concatenate(dstl_tiles, axis=0)
        onehot = (dstl_all[:, None, :] == np.arange(128)[None, :, None])
        per_core.append({
            "idxA": np.concatenate(idxA).astype(np.int16),
            "idxB": np.concatenate(idxB).astype(np.int16),
            "dstl": dstl_all,
            "snm": np.ascontiguousarray(
                onehot.transpose(1, 0, 2)).astype(ml_dtypes.bfloat16),
            "ste": np.ascontiguousarray(
                onehot.transpose(2, 0, 1)).astype(ml_dtypes.bfloat16),
        })
    return sched, per_core


def build_kernel(sched, n_tiles_total, nA, nB):
    from concourse import bass, bacc, tile, mybir

    F32 = mybir.dt.float32
    BF16 = mybir.dt.bfloat16
    I16 = mybir.dt.int16
    AL = mybir.AluOpType
    AF = mybir.ActivationFunctionType

    nc = bacc.Bacc(None, target_bir_lowering=False, num_swdge_queues=4)

    xT_in = nc.dram_tensor("xT", [D, SHARD_PAD], BF16, kind="ExternalInput")
    W_in = [nc.dram_tensor(f"w{l}", [D, D], BF16, kind="ExternalInput") for l in range(3)]
    b_in = [nc.dram_tensor(f"b{l}", [D, 1], F32, kind="ExternalInput") for l in range(3)]
    idxA_in = nc.dram_tensor("idxA", [128, nA // 16], I16, kind="ExternalInput")
    idxB_in = nc.dram_tensor("idxB", [128, nB // 16], I16, kind="ExternalInput")
    dstlc_in = nc.dram_tensor("dstlc", [128, n_tiles_total], F32, kind="ExternalInput")
    dstlr_in = nc.dram_tensor("dstlr", [1, n_tiles_total * BLK], BF16, kind="ExternalInput")
    iota_row_in = nc.dram_tensor("iota_row", [128, 128], BF16, kind="ExternalInput")
    iota_col_in = nc.dram_tensor("iota_col", [128, 1], F32, kind="ExternalInput")
    ones_row_in = nc.dram_tensor("ones_row", [1, 128], BF16, kind="ExternalInput")
    ones_col_in = nc.dram_tensor("ones_col", [128, 1], BF16, kind="ExternalInput")
    ident_in = nc.dram_tensor("ident", [128, 128], BF16, kind="ExternalInput")
    snm_in = nc.dram_tensor("snm", [128, n_tiles_total, 128], BF16, kind="ExternalInput")
    ste_in = nc.dram_tensor("ste", [128, n_tiles_total, 128], BF16, kind="ExternalInput")
    out_dram = nc.dram_tensor("out", [SHARD_PAD, D_OUT], F32, kind="ExternalOutput")

    shard_dram = nc.dram_tensor("shard_d", [SHARD_PAD, D], BF16, kind="Internal")
    table = nc.dram_tensor(
        "table_d", [NCORES * SHARD_PAD, D], BF16, kind="Internal",
        addr_space="Shared")

    chunk_plan = sched["chunk_plan"]
    chunk_tiles = [sum(nt for _, _, nt in plan) for plan in chunk_plan]
    max_chunk_tiles = max(chunk_tiles)
    n_table_rows = NCORES * SHARD_PAD

    with tile.TileContext(nc) as tc:
        with (
            tc.tile_pool(name="const", bufs=1) as cpool,
            tc.tile_pool(name="state", bufs=1) as spool,
            tc.tile_pool(name="hTp", bufs=1) as hTp,
            tc.tile_pool(name="gbuf", bufs=2) as gpool,
            tc.tile_pool(name="work", bufs=3) as wpool,
            tc.tile_pool(name="cols", bufs=2) as colpool,
            tc.tile_pool(name="psBig", bufs=1, space="PSUM") as psBig,
            tc.tile_pool(name="psO", bufs=1, space="PSUM") as psO,
            tc.tile_pool(name="psD", bufs=1, space="PSUM") as psD,
        ):
            iota_row = cpool.tile([128, 128], BF16)
            nc.sync.dma_start(iota_row[:], iota_row_in[:])
            iota_col = cpool.tile([128, 1], F32)
            nc.sync.dma_start(iota_col[:], iota_col_in[:])
            ones_row = cpool.tile([1, 128], BF16)
            nc.sync.dma_start(ones_row[:], ones_row_in[:])
            ones_col = cpool.tile([128, 1], BF16)
            nc.sync.dma_start(ones_col[:], ones_col_in[:])
            ident = cpool.tile([128, 128], BF16)
            nc.sync.dma_start(ident[:], ident_in[:])
            idxA_sb = cpool.tile([128, nA // 16], I16)
            nc.sync.dma_start(idxA_sb[:], idxA_in[:])
            idxB_sb = cpool.tile([128, nB // 16], I16)
            nc.sync.dma_start(idxB_sb[:], idxB_in[:])
            dstlc = cpool.tile([128, n_tiles_total], F32)
            nc.sync.dma_start(dstlc[:], dstlc_in[:])
            Wt, bt = [], []
            for l in range(3):
                w = cpool.tile([D, D], BF16, tag=f"w{l}")
                nc.sync.dma_start(w[:], W_in[l][:])
                Wt.append(w)
                b = cpool.tile([D, 1], F32, tag=f"b{l}")
                nc.sync.dma_start(b[:], b_in[l][:])
                bt.append(b)

            xT = spool.tile([D, SHARD_PAD], BF16, tag="xT")
            nc.sync.dma_start(xT[:], xT_in[:])
            h_nm = spool.tile([128, NBLK, D], BF16, tag="h_nm")
            hnD = spool.tile([128, NBLK, D], BF16, tag="hnD")
            x_nm = spool.tile([128, NBLK, D], BF16, tag="x_nm")

            for layer in range(3):
                # ===== dense =====
                hT = hTp.tile([D, SHARD_PAD], BF16, tag="hT")
                for j in range(0, SHARD_PAD, 512):
                    jn = min(512, SHARD_PAD - j)
                    w_ps = psBig.tile([128, 512], F32, tag="big")
                    nc.tensor.matmul(w_ps[:, 0:jn], Wt[layer][:], xT[:, j:j + jn],
                                     start=True, stop=True)
                    nc.vector.tensor_scalar(
                        hT[:, j:j + jn], w_ps[:, 0:jn], bt[layer][:], None, AL.add)
                for b in range(NBLK):
                    t_ps = psBig.tile([128, 128], BF16, tag="big")
                    nc.tensor.transpose(t_ps[:], hT[:, b * 128:(b + 1) * 128], ident[:])
                    nc.vector.tensor_copy(h_nm[:, b, :], t_ps[:])
                n2c = colpool.tile([128, NBLK], F32, tag="n2c")
                for b in range(NBLK):
                    scr0 = wpool.tile([128, D], BF16, tag="scr0")
                    nc.scalar.activation(scr0[:], h_nm[:, b, :], AF.Square,
                                         accum_out=n2c[:, b:b + 1])
                nc.vector.tensor_scalar_max(n2c[:], n2c[:], 1e-24)
                nrm = colpool.tile([128, NBLK], F32, tag="nrm")
                nc.scalar.activation(nrm[:], n2c[:], AF.Sqrt)
                rnrm = colpool.tile([128, NBLK], F32, tag="rnrm")
                nc.vector.reciprocal(rnrm[:], nrm[:])
                for b in range(NBLK):
                    nc.vector.tensor_scalar(
                        hnD[:, b, :], h_nm[:, b, :], rnrm[:, b:b + 1], None, AL.mult)

                # ===== allgather h shard (i-major contiguous dump) =====
                nc.sync.dma_start(
                    shard_dram[:].rearrange("(i b) d -> i (b d)", i=128),
                    h_nm[:].rearrange("i b d -> i (b d)"))
                nc.gpsimd.collective_compute(
                    "AllGather",
                    mybir.AluOpType.bypass,
                    replica_groups=[list(range(NCORES))],
                    ins=[shard_dram[:].opt()],
                    outs=[table[:].opt()],
                )

                # ===== edge phase =====
                tile_off = 0
                offA = 0
                offB = 0
                qsel = 0
                for ci, plan in enumerate(chunk_plan):
                    T = chunk_tiles[ci]
                    TA = plan[0][2]
                    blk_id = plan[0][0]
                    G = gpool.tile([128, max_chunk_tiles, D], BF16, tag="G")

                    def issue_gathers(idx_sb, off, t0, ntiles, base_row):
                        nonlocal qsel
                        done = 0
                        while done < ntiles:
                            nt = min(GMAX_TILES, ntiles - done)
                            nidx = nt * 128
                            nc.gpsimd.dma_gather(
                                G[:, t0 + done:t0 + done + nt, :],
                                table[base_row:n_table_rows, :],
                                idx_sb[:, (off + done * 128) // 16:
                                       (off + (done + nt) * 128) // 16],
                                nidx, nidx, D,
                                single_packet=True,
                                queue_num=qsel % 4,
                            )
                            qsel += 1
                            done += nt

                    issue_gathers(idxA_sb, offA, 0, TA, 0)
                    if T - TA > 0 and n_table_rows > SPLIT:
                        issue_gathers(idxB_sb, offB, TA, T - TA, SPLIT)
                    offA += TA * 128
                    offB += (T - TA) * 128

                    Q = colpool.tile([128, max_chunk_tiles], F32, tag="Q")
                    N2 = colpool.tile([128, max_chunk_tiles], F32, tag="N2")

                    snm_sb = gpool.tile([128, max_chunk_tiles, 128], BF16, tag="snm")
                    nc.sync.dma_start(
                        snm_sb[:, 0:T, :], snm_in[:, tile_off:tile_off + T, :])
                    ste_sb = gpool.tile([128, max_chunk_tiles, 128], BF16, tag="ste")
                    nc.sync.dma_start(
                        ste_sb[:, 0:T, :], ste_in[:, tile_off:tile_off + T, :])
                    hd_ps = psBig.tile([128, max_chunk_tiles * 128], F32, tag="big")
                    for t in range(T):
                        nc.tensor.matmul(
                            hd_ps[:, t * 128:(t + 1) * 128], snm_sb[:, t, :],
                            hnD[:, blk_id, :], start=True, stop=True)
                    U = gpool.tile([128, max_chunk_tiles, 128], BF16, tag="U")
                    nc.vector.tensor_tensor(
                        U[:, 0:T, :].rearrange("p t d -> p (t d)"),
                        hd_ps[:, 0:T * 128],
                        G[:, 0:T, :].rearrange("p t d -> p (t d)"), AL.add)
                    squ = gpool.tile([128, max_chunk_tiles, 128], BF16, tag="squ")
                    nc.scalar.activation(
                        squ[:, 0:T, :].rearrange("p t d -> p (t d)"),
                        U[:, 0:T, :].rearrange("p t d -> p (t d)"), AF.Square)
                    nc.vector.tensor_reduce(
                        Q[:, 0:T], squ[:, 0:T, :], mybir.AxisListType.X, AL.add)
                    sqg = gpool.tile([128, max_chunk_tiles, 128], BF16, tag="sqg")
                    nc.scalar.activation(
                        sqg[:, 0:T, :].rearrange("p t d -> p (t d)"),
                        G[:, 0:T, :].rearrange("p t d -> p (t d)"), AF.Square)
                    nc.vector.tensor_reduce(
                        N2[:, 0:T], sqg[:, 0:T, :], mybir.AxisListType.X, AL.add)

                    # alpha = (Q - N2 - 1) * 0.5 / sqrt(N2); EX = exp(alpha)
                    n2cl = colpool.tile([128, max_chunk_tiles], F32, tag="n2cl")
                    nc.vector.tensor_scalar_max(n2cl[:, 0:T], N2[:, 0:T], 1e-24)
                    nsr = colpool.tile([128, max_chunk_tiles], F32, tag="nsr")
                    nc.scalar.activation(nsr[:, 0:T], n2cl[:, 0:T], AF.Sqrt)
                    rn = colpool.tile([128, max_chunk_tiles], F32, tag="rn")
                    nc.vector.reciprocal(rn[:, 0:T], nsr[:, 0:T])
                    al1 = colpool.tile([128, max_chunk_tiles], F32, tag="al1")
                    nc.vector.tensor_tensor(al1[:, 0:T], Q[:, 0:T], N2[:, 0:T],
                                            AL.subtract)
                    nc.vector.tensor_scalar(al1[:, 0:T], al1[:, 0:T], -1.0, 0.5,
                                            AL.add, AL.mult)
                    nc.vector.tensor_tensor(al1[:, 0:T], al1[:, 0:T], rn[:, 0:T],
                                            AL.mult)
                    EX = colpool.tile([128, max_chunk_tiles], F32, tag="EX")
                    nc.scalar.activation(EX[:, 0:T], al1[:, 0:T], AF.Exp)

                    blk_ps = psO.tile([128, 128], F32, tag="blk")
                    den_ps = psD.tile([128, 1], F32, tag="den")
                    for t in range(T):
                        W_T = wpool.tile([128, 128], BF16, tag="W_T")
                        nc.vector.tensor_scalar(
                            W_T[:], ste_sb[:, t, :], EX[:, t:t + 1], None, AL.mult)
                        first = t == 0
                        last = t == T - 1
                        nc.tensor.matmul(blk_ps[:], W_T[:], G[:, t, :],
                                         start=first, stop=last)
                        nc.tensor.matmul(den_ps[:], W_T[:], ones_col[:],
                                         start=first, stop=last)

                    den = colpool.tile([128, 1], F32, tag="den")
                    nc.vector.tensor_scalar_max(den[:], den_ps[:], 1e-30)
                    rd = colpool.tile([128, 1], F32, tag="rd")
                    nc.vector.reciprocal(rd[:], den[:])
                    if layer < 2:
                        nc.scalar.activation(
                            x_nm[:, blk_id, :], blk_ps[:, 0:128], AF.Relu,
                            scale=rd[:])
                    else:
                        z = wpool.tile([128, D_OUT], F32, tag="z")
                        nc.scalar.activation(
                            z[:], blk_ps[:, 0:D_OUT], AF.Copy, scale=rd[:])
                        mx = colpool.tile([128, 1], F32, tag="mx")
                        nc.vector.tensor_reduce(
                            mx[:], z[:], mybir.AxisListType.X, AL.max)
                        nmx = colpool.tile([128, 1], F32, tag="nmx")
                        nc.vector.tensor_scalar_mul(nmx[:], mx[:], -1.0)
                        escr = wpool.tile([128, D_OUT], F32, tag="escr")
                        se = colpool.tile([128, 1], F32, tag="se")
                        nc.scalar.activation(escr[:], z[:], AF.Exp,
                                             bias=nmx[:], accum_out=se[:])
                        lse = colpool.tile([128, 1], F32, tag="lse")
                        nc.scalar.activation(lse[:], se[:], AF.Ln)
                        zo = wpool.tile([128, D_OUT], F32, tag="zo")
                        nc.vector.tensor_scalar(
                            zo[:], z[:], mx[:], lse[:], AL.subtract, AL.subtract)
                        nc.sync.dma_start(
                            out_dram[:].rearrange("(b i) d -> b i d", i=128)[blk_id],
                            zo[:])
                    tile_off += T

                if layer < 2:
                    for b in range(NBLK):
                        t_ps = psBig.tile([128, 128], BF16, tag="big")
                        nc.tensor.transpose(t_ps[:], x_nm[:, b, :], ident[:])
                        nc.vector.tensor_copy(xT[:, b * 128:(b + 1) * 128], t_ps[:])

    nc.finalize()
    return nc


def prepare(x, edge_index, W1, b1, W2, b2, W3, b3):
    x = np.asarray(x, dtype=np.float32)
    sched, per_core = preprocess(x, edge_index)
    tiles = sched["tiles"]
    n_tiles_total = int(tiles.sum())
    nA = int(tiles[:, 0].sum()) * BLK
    nB = max(int(tiles[:, 1].sum()) * BLK, 128)

    nc = build_kernel(sched, n_tiles_total, nA, nB)

    def wt_pad(W):
        W = np.asarray(W, np.float32)
        dout, din = W.shape
        M = np.zeros((D, D), np.float32)
        M[:din, :dout] = W.T
        return M.astype(ml_dtypes.bfloat16)

    def b_pad(b):
        b = np.asarray(b, np.float32)
        M = np.zeros((D, 1), np.float32)
        M[: len(b), 0] = b
        return M

    Ws = [wt_pad(W1), wt_pad(W2), wt_pad(W3)]
    bs = [b_pad(b1), b_pad(b2), b_pad(b3)]

    iota_row = np.tile(np.arange(128, dtype=np.float32)[None, :], (128, 1)).astype(
        ml_dtypes.bfloat16)
    iota_col = np.arange(128, dtype=np.float32)[:, None]
    ones_row = np.ones((1, 128), ml_dtypes.bfloat16)
    ones_col = np.ones((128, 1), ml_dtypes.bfloat16)
    ident = np.eye(128, dtype=np.float32).astype(ml_dtypes.bfloat16)

    in_maps = []
    for cid in range(NCORES):
        pc = per_core[cid]
        xs = np.zeros((D, SHARD_PAD), np.float32)
        xs[:, :SHARD] = x[cid * SHARD:(cid + 1) * SHARD].T
        dstl = pc["dstl"]
        idxB = pc["idxB"]
        if len(idxB) < nB:
            idxB = np.concatenate([idxB, np.zeros(nB - len(idxB), np.int16)])
        m = {
            "xT": xs.astype(ml_dtypes.bfloat16),
            "w0": Ws[0], "w1": Ws[1], "w2": Ws[2],
            "b0": bs[0], "b1": bs[1], "b2": bs[2],
            "idxA": _wrap_idx(pc["idxA"]),
            "idxB": _wrap_idx(idxB),
            "dstlc": np.ascontiguousarray(dstl.T.astype(np.float32)),
            "dstlr": dstl.reshape(1, -1).astype(np.float32).astype(
                ml_dtypes.bfloat16),
            "snm": pc["snm"],
            "ste": pc["ste"],
            "iota_row": iota_row,
            "iota_col": iota_col,
            "ones_row": ones_row,
            "ones_col": ones_col,
            "ident": ident,
        }
        in_maps.append(m)
    return nc, in_maps


def _ensure_ntff_shim():
    """bass_utils trace=True needs antenv.axon_hooks; provide it if absent."""
    import types, importlib
    try:
        importlib.import_module("antenv.axon_hooks")
        return
    except Exception:
        pass
    mod = types.ModuleType("antenv.axon_hooks")
    _h = [None]
    mod.set_axon_ntff_profile_hook = lambda h: _h.__setitem__(0, h)
    mod.get_axon_ntff_profile_hook = lambda: _h[0]
    sys.modules["antenv.axon_hooks"] = mod
    try:
        import antenv
        antenv.axon_hooks = mod
        from trn_agent_boot.trn_boot import _ntff_profile_via_ctypes
        mod.set_axon_ntff_profile_hook(
            _ntff_profile_via_ctypes("/opt/axon/libaxon_pjrt.so"))
    except Exception:
        pass


def _axon_reset():
    try:
        import ctypes, jax
        jax.devices()
        lib = ctypes.CDLL("/opt/axon/libaxon_pjrt.so")
        lib.axon_reset.restype = ctypes.c_int64
        lib.axon_reset()
    except Exception:
        pass


def _run_once(x, edge_index, W1, b1, W2, b2, W3, b3, trace=True):
    _ensure_ntff_shim()
    from concourse import bass_utils

    nc, in_maps = prepare(x, edge_index, W1, b1, W2, b2, W3, b3)
    _axon_reset()
    res = bass_utils.run_bass_kernel_spmd(
        nc, in_maps, core_ids=list(range(NCORES)), trace=trace)
    out = np.zeros((N_NODES, D_OUT), np.float32)
    for cid in range(NCORES):
        out[cid * SHARD:(cid + 1) * SHARD] = res.results[cid]["out"][:SHARD]
    return out, res.exec_time_ns


def kernel(x, edge_index, W1, b1, W2, b2, W3, b3):
    import subprocess, tempfile, os
    try:
        out, t = _run_once(x, edge_index, W1, b1, W2, b2, W3, b3)
        kernel.last_exec_time_ns = t
        return out
    except Exception:
        pass
    # device/client wedged: retry in fresh subprocesses
    for attempt in range(3):
        with tempfile.TemporaryDirectory() as td:
            fin = os.path.join(td, "in.npz")
            fout = os.path.join(td, "out.npz")
            np.savez(fin, x=x, edge_index=np.asarray(edge_index),
                     W1=W1, b1=b1, W2=W2, b2=b2, W3=W3, b3=b3)
            r = subprocess.run(
                [sys.executable, os.path.abspath(__file__), "--worker", fin, fout],
                timeout=3600)
            if r.returncode == 0 and os.path.exists(fout):
                d = np.load(fout)
                kernel.last_exec_time_ns = (
                    int(d["t"]) if int(d["t"]) >= 0 else None)
                return d["out"]
    raise RuntimeError("kernel failed after subprocess retries")


kernel.last_exec_time_ns = None


if __name__ == "__main__" and len(sys.argv) == 4 and sys.argv[1] == "--worker":
    d = np.load(sys.argv[2])
    out, t = _run_once(d["x"], d["edge_index"], d["W1"], d["b1"],
                       d["W2"], d["b2"], d["W3"], d["b3"])
    np.savez(sys.argv[3], out=out, t=(t if t is not None else -1))
